# revision 4
# baseline (speedup 1.0000x reference)
"""Trainium2 Bass kernel v2: sparse (top-k) causal attention, data-parallel over batch.

Reference semantics (B=32, H=8, S=512, D=64, k_index=5):
  S_raw = (Q @ K^T) / sqrt(d_k), causal-masked
  P     = softmax(S_raw)
  rows >= k_index: keep only P >= (k_index-th largest of row)
  W     = softmax(P');  W[row 0] = 0;  out = W @ V

Key structure (per core: 32 heads x 4 causal q-tiles of 128 rows):
  - QK^T as two K=128 fp16 matmuls: [qh;ql]@[kh;kh] + [qh;ql]@[kl;kl]
    (q=qh+ql, k=kh+kl fp16 splits -> ~2^-22 score accuracy at 16-bit PE
    rate). First matmul start=True: non-accumulating PSUM writes stream
    2x faster than accumulating ones; mask matmul accumulates last.
  - e = exp(s/8) on ACT with row-sum z accumulated (no max-subtraction
    needed; scores ~ N(0,1)).
  - em = e/z via gpsimd normalize_recip (the only fast Pool op), so the
    second exp runs at full ACT rate with immediate scale (tensor-scale
    activations run at half rate).
  - u = exp(em); w = (e >= top8[k-1]) * u on DVE -> bf16.
  - W^T via PE transposes, stage-interleaved across the head pair so
    consecutive transposes alternate PSUM banks (56ns vs 107ns).
  - mm2 rhs = [V | 1]: the ones column makes each o_ps row also carry
    z2 = sum_kept(u), including the (S-128) tail mass for rows<k_index
    via the rank-k ones matmuls. One reciprocal + one broadcasted
    tensor_tensor per head scales all 4 tiles at once.
  - rows < k_index keep everything (thr=-1); row 0 zeroed via rz2=0.

Sharding: batch 32 -> 4 per core across 8 cores; each (b,h) independent.
Host packs one [bh, 128, 1792] uint16 tile per head: fp16 [qh;ql],
[kh;kh], [kl;kl] pre-transposed plus bf16 [V|1|0] -> single input DMA.
"""

import math

import numpy as np
import ml_dtypes

import concourse.bass as bass
import concourse.bacc as bacc
import concourse.mybir as mybir
import concourse.tile as tile
from concourse.bass_utils import run_bass_kernel_spmd
from concourse.masks import make_causal_mask, make_identity

N_CORES = 8
F32 = mybir.dt.float32
BF16 = mybir.dt.bfloat16
FP16 = mybir.dt.float16

# test.py hooks
TRACE = False
LAST_RESULT = None
BH_OVERRIDE = None  # dev only: limit (b,h) pairs per core

_NC_CACHE = {}

VW = 64  # V columns (z2 comes from the stt accumulator)


def _build(bh_count: int, S: int, D: int, d_k: int, k_index: int) -> bass.Bass:
    P = 128
    NT = S // P
    KI = k_index
    NEG = -1.0e5
    scale = 1.0 / math.sqrt(float(d_k))
    assert 1 <= KI <= 8 and S % P == 0 and D <= P

    QKV_W = 3 * S + NT * VW  # fp16 cols: qh|ql, kh|kh, kl|kl, then bf16 [V|1|0]

    nc = bacc.Bacc("TRN2", target_bir_lowering=False, debug=False)
    qkv = nc.declare_dram_parameter("qkv", [bh_count, P, QKV_W], FP16, isOutput=False)
    out = nc.declare_dram_parameter("out", [bh_count, S, D], F32, isOutput=True)

    with tile.TileContext(nc) as tc:
        with (
            tc.tile_pool(name="const", bufs=1) as cpool,
            tc.tile_pool(name="inp", bufs=10) as ipool,
            tc.tile_pool(name="big", bufs=10) as bpool,
            tc.tile_pool(name="wbuf", bufs=10) as wpool,
            tc.tile_pool(name="wt", bufs=10) as wtpool,
            tc.tile_pool(name="stat", bufs=20) as spool,
            tc.tile_pool(name="obuf", bufs=6) as opool,
            tc.tile_pool(name="ps_s", bufs=2, space="PSUM") as ps_s,
            tc.tile_pool(name="ps_o", bufs=4, space="PSUM") as ps_o,
            tc.tile_pool(name="ps_wt", bufs=2, space="PSUM") as ps_wt,
        ):
            # constants
            mask_f = cpool.tile([P, P], F32)
            make_causal_mask(nc, mask_f[:, :], mask_val=NEG)
            mask_b = cpool.tile([P, P], BF16)
            nc.vector.tensor_copy(mask_b[:, :], mask_f[:, :])
            ident_f = cpool.tile([P, P], F32)
            make_identity(nc, ident_f[:, :])
            ident_b = cpool.tile([P, P], BF16)
            nc.vector.tensor_copy(ident_b[:, :], ident_f[:, :])
            ones_k = cpool.tile([P, KI], BF16)
            nc.vector.memset(ones_k[:, :], 1.0)

            st = {}  # per-iteration tile state
            bh_state = {}  # per-head tiles

            def s_dma(i):
                bh, t = divmod(i, NT)
                if t:
                    return
                qk = ipool.tile([P, QKV_W], FP16, tag="qkv", name=f"qkv_{bh}")
                nc.sync.dma_start(qk[:, :], qkv[bh])
                o_all = opool.tile([P, NT, D], F32, tag="o_all", name=f"oall_{bh}")
                rz2 = spool.tile([P, NT], F32, tag="rz2", name=f"rz2_{bh}")
                o_ps_t = ps_o.tile([P, NT, VW], F32, tag="o", name=f"ops_{bh}")
                bh_state[bh] = (qk, o_all, rz2, o_ps_t, 0)

            def s_mm1(i):
                bh, t = divmod(i, NT)
                C = P * (t + 1)
                qk = bh_state[bh][0]
                s_ps = ps_s.tile([P, S], F32, tag="s", name=f"sps_{i}")
                # fresh (non-accumulating) write first: 2x PSUM rate
                nc.tensor.matmul(
                    s_ps[:, :C], lhsT=qk[:, bass.ts(t, P)],
                    rhs=qk[:, S : S + C], start=True, stop=False)
                nc.tensor.matmul(
                    s_ps[:, :C], lhsT=qk[:, bass.ts(t, P)],
                    rhs=qk[:, 2 * S : 2 * S + C], start=False, stop=False)
                nc.tensor.matmul(
                    s_ps[:, bass.ts(t, P)],
                    lhsT=ident_b[:, :], rhs=mask_b[:, :],
                    start=False, stop=True)
                st[i] = {"s_ps": s_ps}

            def s_exp1(i):
                bh, t = divmod(i, NT)
                C = P * (t + 1)
                d = st[i]
                e_s = bpool.tile([P, S], F32, tag="e", name=f"e_{i}")
                z = spool.tile([P, 1], F32, tag="z", name=f"z_{i}")
                nc.scalar.activation(
                    e_s[:, :C],
                    d["s_ps"][:, :C],
                    mybir.ActivationFunctionType.Exp,
                    scale=scale,
                    accum_out=z[:, :],
                )
                d["e"], d["z"] = e_s, z

            def s_top8(i):
                bh, t = divmod(i, NT)
                C = P * (t + 1)
                d = st[i]
                top8 = spool.tile([P, 8], F32, tag="top8", name=f"top8_{i}")
                nc.vector.max(out=top8[:, :], in_=d["e"][:, :C])
                if t == 0:
                    nc.vector.memset(top8[0:KI, KI - 1 : KI], -1.0)
                d["top8"] = top8

            def s_em(i):
                bh, t = divmod(i, NT)
                C = P * (t + 1)
                d = st[i]
                em = bpool.tile([P, S], F32, tag="em", name=f"em_{i}")
                nc.gpsimd.normalize_recip(em[:, :C], d["e"][:, :C], d["z"][:, :])
                d["em"] = em

            def s_exp2(i):
                bh, t = divmod(i, NT)
                C = P * (t + 1)
                d = st[i]
                u_s = bpool.tile([P, S], F32, tag="u", name=f"u_{i}")
                nc.scalar.activation(
                    u_s[:, :C],
                    d["em"][:, :C],
                    mybir.ActivationFunctionType.Exp,
                )
                d["u"] = u_s

            def s_stt(i):
                bh, t = divmod(i, NT)
                C = P * (t + 1)
                d = st[i]
                rz2 = bh_state[bh][2]
                w_s = wpool.tile([P, S], BF16, tag="w", name=f"w_{i}")
                nc.vector.scalar_tensor_tensor(
                    out=w_s[:, :C],
                    in0=d["e"][:, :C],
                    scalar=d["top8"][:, KI - 1 : KI],
                    in1=d["u"][:, :C],
                    op0=mybir.AluOpType.is_ge,
                    op1=mybir.AluOpType.mult,
                    accum_out=rz2[:, t : t + 1],
                )
                if t == 0:
                    nc.vector.tensor_scalar_add(
                        rz2[0:KI, 0:1], rz2[0:KI, 0:1], float(S - P)
                    )
                d["w"] = w_s

            def s_tr_alloc(i):
                d = st[i]
                d["wtp"] = ps_wt.tile([P, NT, P], BF16, tag="wtp", name=f"wtp_{i}")

            def s_tr_chunk(i, c):
                d = st[i]
                nc.tensor.transpose(
                    d["wtp"][:, c, :], d["w"][:, bass.ts(c, P)], ident_b[:, :]
                )

            def s_ev(i):
                bh, t = divmod(i, NT)
                d = st[i]
                wt_s = wtpool.tile([P, NT, P], BF16, tag="wt", name=f"wt_{i}")
                nc.vector.tensor_copy(wt_s[:, 0 : t + 1, :], d["wtp"][:, 0 : t + 1, :])
                d["wt"] = wt_s

            def s_mm2_chunk(i, c):
                bh, t = divmod(i, NT)
                d = st[i]
                qk, _, _, o_ps_t, half = bh_state[bh]
                v0 = 3 * S  # fp16-col offset of the bf16 V block
                nc.tensor.matmul(
                    o_ps_t[:, t, :],
                    lhsT=d["wt"][:, c, :],
                    rhs=qk[:, v0 + c * VW : v0 + (c + 1) * VW].bitcast(BF16),
                    start=(c == 0),
                    stop=(c == t and t > 0),
                )
                if t == 0:
                    for cc in range(1, NT):
                        nc.tensor.matmul(
                            o_ps_t[0:KI, 0, :],
                            lhsT=ones_k[:, 0:KI],
                            rhs=qk[:, v0 + cc * VW : v0 + (cc + 1) * VW].bitcast(BF16),
                            start=False,
                            stop=(cc == NT - 1),
                        )

            def s_osc(i):
                bh, t = divmod(i, NT)
                st.pop(i)
                if t != NT - 1:
                    return
                qk, o_all, rz2, o_ps_t, half = bh_state[bh]
                nc.vector.reciprocal(rz2[:, :], rz2[:, :])
                nc.vector.memset(rz2[0:1, 0:1], 0.0)
                nc.vector.tensor_tensor(
                    out=o_all[:, :, :],
                    in0=o_ps_t[:, :, 0:D],
                    in1=rz2[:, :, None].broadcast_to([P, NT, D]),
                    op=mybir.AluOpType.mult,
                )
                nc.sync.dma_start(
                    out[bh].rearrange("(c p) d -> p c d", p=P), o_all[:, :, :]
                )
                del bh_state[bh]

            # G=2 head interleave, stage-major emission so paired heads'
            # PE ops alternate PSUM banks (non-conflicting writes stream
            # 2x) and the Tile scheduler can pipeline across engines.
            G = 4
            for g0 in range(0, bh_count, G):
                members = list(range(g0, min(g0 + G, bh_count)))
                for bh in members:
                    s_dma(bh * NT)
                for t in range(NT):
                    ii = [bh * NT + t for bh in members]
                    for fn in (s_mm1, s_exp1, s_top8, s_em, s_exp2, s_stt):
                        for i in ii:
                            fn(i)
                    # tr/ev/mm2 in sub-pairs: wtp pool has 2 bufs, so only
                    # two heads' transposes can alternate PSUM banks at once
                    for j0 in range(0, len(ii), 2):
                        sub = ii[j0 : j0 + 2]
                        for i in sub:
                            s_tr_alloc(i)
                        for c in range(t + 1):
                            for i in sub:
                                s_tr_chunk(i, c)
                        for i in sub:
                            s_ev(i)
                        for c in range(t + 1):
                            for i in sub:
                                s_mm2_chunk(i, c)
                    for i in ii:
                        s_osc(i)
    nc.compile()
    return nc


def _get_nc(bh_count, S, D, d_k, k_index):
    key = (bh_count, S, D, d_k, k_index)
    if key not in _NC_CACHE:
        _NC_CACHE[key] = _build(bh_count, S, D, d_k, k_index)
    return _NC_CACHE[key]


def _numpy_fallback(q, k, v, mask, d_k, k_index):
    """Straight port of the reference for inputs the Bass kernel doesn't
    cover (non-causal mask / incompatible shapes). Slow but correct."""
    NEG = np.float32(-1e32)
    b, h, s, _ = q.shape
    scores = np.einsum("bhqd,bhkd->bhqk", q, k) / np.sqrt(np.float32(d_k))
    scores = np.where(mask == 0, NEG, scores)
    scores = scores - scores.max(axis=-1, keepdims=True)
    e = np.exp(scores)
    scores = e / e.sum(axis=-1, keepdims=True)
    sa = scores[:, :, :k_index, :]
    sb = scores[:, :, k_index:, :].reshape(b * h * (s - k_index), s)
    srt = -np.sort(-sb, axis=-1)
    thr = srt[:, k_index - 1 : k_index]
    sb = np.where(sb - thr >= 0, sb, NEG)
    sb = sb.reshape(b, h, s - k_index, s)
    scores = np.concatenate([sa, sb], axis=2)
    scores = scores - scores.max(axis=-1, keepdims=True)
    e = np.exp(scores)
    scores = e / e.sum(axis=-1, keepdims=True)
    scores[:, :, 0, :] = 0.0
    return np.einsum("bhqk,bhkd->bhqd", scores, v).astype(np.float32)


def _is_causal(mask, S):
    if mask is None:
        return True
    m = np.asarray(mask)
    if m.size != S * S:
        return False
    return bool(np.array_equal(m.reshape(S, S) != 0, np.tril(np.ones((S, S), bool))))


def kernel(q, k, v, mask=None, d_k=None, k_index=None, **_unused):
    global LAST_RESULT
    q = np.asarray(q, dtype=np.float32)
    k = np.asarray(k, dtype=np.float32)
    v = np.asarray(v, dtype=np.float32)
    B, H, S, D = q.shape
    d_k = int(d_k) if d_k is not None else D
    k_index = int(k_index) if k_index is not None else 5

    if (
        B % N_CORES != 0
        or S % 128 != 0
        or D != 64
        or not (1 <= k_index <= 8)
        or not _is_causal(mask, S)
    ):
        mask_np = (
            np.asarray(mask)
            if mask is not None
            else np.tril(np.ones((S, S), np.int32))[None, None]
        )
        return _numpy_fallback(q, k, v, mask_np, d_k, k_index)

    P = 128
    NT = S // P
    bpc = B // N_CORES
    bh_full = bpc * H
    bh_count = BH_OVERRIDE or bh_full

    qT = np.transpose(q, (0, 1, 3, 2))  # [B, H, D, S]
    kT = np.transpose(k, (0, 1, 3, 2))
    qh = qT.astype(np.float16)
    ql = (qT - qh.astype(np.float32)).astype(np.float16)
    kh = kT.astype(np.float16)
    kl = (kT - kh.astype(np.float32)).astype(np.float16)
    # fp16 rows packed [qh;ql | kh;kh | kl;kl]: [B,H,128,3S] viewed per-bh
    qk16 = np.concatenate(
        [
            np.concatenate([qh, ql], axis=2),
            np.concatenate([kh, kh], axis=2),
            np.concatenate([kl, kl], axis=2),
        ],
        axis=3,
    )  # [B, H, 2D=128, 3S]
    # bf16 [V|1|0] block: [B,H,128,NT*VW]
    vb = np.zeros((B, H, P, NT * VW), dtype=ml_dtypes.bfloat16)
    vr = v.reshape(B, H, NT, P, D).transpose(0, 1, 3, 2, 4)  # [B,H,P,NT,D]
    vbl = vb.reshape(B, H, P, NT, VW)
    vbl[..., 0:D] = vr.astype(ml_dtypes.bfloat16)
    qkv_h = np.concatenate(
        [qk16.view(np.uint16), vb.view(np.uint16).reshape(B, H, P, NT * VW)],
        axis=3,
    ).view(np.float16)  # [B, H, 128, 3S + NT*VW]

    nc = _get_nc(bh_count, S, D, d_k, k_index)

    in_maps = []
    for i in range(N_CORES):
        sl = slice(i * bpc, (i + 1) * bpc)
        in_maps.append(
            {
                "qkv": np.ascontiguousarray(
                    qkv_h[sl].reshape(bh_full, P, qkv_h.shape[3])[:bh_count]
                ),
            }
        )

    res = run_bass_kernel_spmd(
        nc, in_maps, core_ids=list(range(N_CORES)), trace=TRACE
    )
    LAST_RESULT = res

    outs = [
        np.asarray(res.results[i]["out"], dtype=np.float32) for i in range(N_CORES)
    ]
    if bh_count != bh_full:
        outs = [
            np.concatenate(
                [o, np.zeros((bh_full - bh_count, S, D), np.float32)], axis=0
            )
            for o in outs
        ]
    return np.concatenate([o.reshape(bpc, H, S, D) for o in outs], axis=0)


# revision 5
# speedup vs baseline: 1.0218x; 1.0218x over previous
"""Trainium2 Bass kernel v2: sparse (top-k) causal attention, data-parallel over batch.

Reference semantics (B=32, H=8, S=512, D=64, k_index=5):
  S_raw = (Q @ K^T) / sqrt(d_k), causal-masked
  P     = softmax(S_raw)
  rows >= k_index: keep only P >= (k_index-th largest of row)
  W     = softmax(P');  W[row 0] = 0;  out = W @ V

Key structure (per core: 32 heads x 4 causal q-tiles of 128 rows):
  - QK^T as two K=128 fp16 matmuls: [qh;ql]@[kh;kh] + [qh;ql]@[kl;kl]
    (q=qh+ql, k=kh+kl fp16 splits -> ~2^-22 score accuracy at 16-bit PE
    rate). First matmul start=True: non-accumulating PSUM writes stream
    2x faster than accumulating ones; mask matmul accumulates last.
  - e = exp(s/8) on ACT with row-sum z accumulated (no max-subtraction
    needed; scores ~ N(0,1)).
  - em = e/z via gpsimd normalize_recip (the only fast Pool op), so the
    second exp runs at full ACT rate with immediate scale (tensor-scale
    activations run at half rate).
  - u = exp(em); w = (e >= top8[k-1]) * u on DVE -> bf16.
  - W^T via PE transposes, stage-interleaved across the head pair so
    consecutive transposes alternate PSUM banks (56ns vs 107ns).
  - mm2 rhs = [V | 1]: the ones column makes each o_ps row also carry
    z2 = sum_kept(u), including the (S-128) tail mass for rows<k_index
    via the rank-k ones matmuls. One reciprocal + one broadcasted
    tensor_tensor per head scales all 4 tiles at once.
  - rows < k_index keep everything (thr=-1); row 0 zeroed via rz2=0.

Sharding: batch 32 -> 4 per core across 8 cores; each (b,h) independent.
Host packs one [bh, 128, 1792] uint16 tile per head: fp16 [qh;ql],
[kh;kh], [kl;kl] pre-transposed plus bf16 [V|1|0] -> single input DMA.
"""

import math

import numpy as np
import ml_dtypes

import concourse.bass as bass
import concourse.bacc as bacc
import concourse.mybir as mybir
import concourse.tile as tile
from concourse.bass_utils import run_bass_kernel_spmd
from concourse.masks import make_causal_mask, make_identity

N_CORES = 8
F32 = mybir.dt.float32
BF16 = mybir.dt.bfloat16
FP16 = mybir.dt.float16

# test.py hooks
TRACE = False
LAST_RESULT = None
BH_OVERRIDE = None  # dev only: limit (b,h) pairs per core

_NC_CACHE = {}

VW = 64  # V columns (z2 comes from the stt accumulator)


def _build(bh_count: int, S: int, D: int, d_k: int, k_index: int) -> bass.Bass:
    P = 128
    NT = S // P
    KI = k_index
    NEG = -1.0e5
    scale = 1.0 / math.sqrt(float(d_k))
    assert 1 <= KI <= 8 and S % P == 0 and D <= P

    QKV_W = 3 * S + NT * VW  # fp16 cols: qh|ql, kh|kh, kl|kl, then bf16 [V|1|0]

    nc = bacc.Bacc("TRN2", target_bir_lowering=False, debug=False)
    qkv = nc.declare_dram_parameter("qkv", [bh_count, P, QKV_W], FP16, isOutput=False)
    out = nc.declare_dram_parameter("out", [bh_count, S, D], F32, isOutput=True)

    with tile.TileContext(nc) as tc:
        with (
            tc.tile_pool(name="const", bufs=1) as cpool,
            tc.tile_pool(name="inp", bufs=8) as ipool,
            tc.tile_pool(name="big", bufs=8) as bpool,
            tc.tile_pool(name="wbuf", bufs=8) as wpool,
            tc.tile_pool(name="wt", bufs=8) as wtpool,
            tc.tile_pool(name="stat", bufs=12) as spool,
            tc.tile_pool(name="obuf", bufs=5) as opool,
            tc.tile_pool(name="ps_s", bufs=2, space="PSUM") as ps_s,
            tc.tile_pool(name="ps_o", bufs=4, space="PSUM") as ps_o,
            tc.tile_pool(name="ps_wt", bufs=2, space="PSUM") as ps_wt,
        ):
            # constants
            mask_f = cpool.tile([P, P], F32)
            make_causal_mask(nc, mask_f[:, :], mask_val=NEG)
            mask_b = cpool.tile([P, P], BF16)
            nc.vector.tensor_copy(mask_b[:, :], mask_f[:, :])
            ident_f = cpool.tile([P, P], F32)
            make_identity(nc, ident_f[:, :])
            ident_b = cpool.tile([P, P], BF16)
            nc.vector.tensor_copy(ident_b[:, :], ident_f[:, :])
            ones_k = cpool.tile([P, KI], BF16)
            nc.vector.memset(ones_k[:, :], 1.0)

            st = {}  # per-iteration tile state
            bh_state = {}  # per-head tiles

            def s_dma(i):
                bh, t = divmod(i, NT)
                if t:
                    return
                qk = ipool.tile([P, QKV_W], FP16, tag="qkv", name=f"qkv_{bh}")
                nc.sync.dma_start(qk[:, :], qkv[bh])
                o_all = opool.tile([P, NT, D], F32, tag="o_all", name=f"oall_{bh}")
                rz2 = spool.tile([P, NT], F32, tag="rz2", name=f"rz2_{bh}")
                o_ps_t = ps_o.tile([P, NT, VW], F32, tag="o", name=f"ops_{bh}")
                bh_state[bh] = (qk, o_all, rz2, o_ps_t, 0)

            def s_mm1(i):
                bh, t = divmod(i, NT)
                C = P * (t + 1)
                qk = bh_state[bh][0]
                s_ps = ps_s.tile([P, S], F32, tag="s", name=f"sps_{i}")
                # fresh (non-accumulating) write first: 2x PSUM rate
                nc.tensor.matmul(
                    s_ps[:, :C], lhsT=qk[:, bass.ts(t, P)],
                    rhs=qk[:, S : S + C], start=True, stop=False)
                nc.tensor.matmul(
                    s_ps[:, :C], lhsT=qk[:, bass.ts(t, P)],
                    rhs=qk[:, 2 * S : 2 * S + C], start=False, stop=False)
                nc.tensor.matmul(
                    s_ps[:, bass.ts(t, P)],
                    lhsT=ident_b[:, :], rhs=mask_b[:, :],
                    start=False, stop=True)
                st[i] = {"s_ps": s_ps}

            def s_exp1(i):
                bh, t = divmod(i, NT)
                C = P * (t + 1)
                d = st[i]
                e_s = bpool.tile([P, S], F32, tag="e", name=f"e_{i}")
                z = spool.tile([P, 1], F32, tag="z", name=f"z_{i}")
                nc.scalar.activation(
                    e_s[:, :C],
                    d["s_ps"][:, :C],
                    mybir.ActivationFunctionType.Exp,
                    scale=scale,
                    accum_out=z[:, :],
                )
                d["e"], d["z"] = e_s, z

            def s_top8(i):
                bh, t = divmod(i, NT)
                C = P * (t + 1)
                d = st[i]
                top8 = spool.tile([P, 8], F32, tag="top8", name=f"top8_{i}")
                nc.vector.max(out=top8[:, :], in_=d["e"][:, :C])
                if t == 0:
                    nc.vector.memset(top8[0:KI, KI - 1 : KI], -1.0)
                d["top8"] = top8

            def s_em(i):
                bh, t = divmod(i, NT)
                C = P * (t + 1)
                d = st[i]
                em = bpool.tile([P, S], F32, tag="em", name=f"em_{i}")
                nc.gpsimd.normalize_recip(em[:, :C], d["e"][:, :C], d["z"][:, :])
                d["em"] = em

            def s_exp2(i):
                bh, t = divmod(i, NT)
                C = P * (t + 1)
                d = st[i]
                u_s = bpool.tile([P, S], F32, tag="u", name=f"u_{i}")
                nc.scalar.activation(
                    u_s[:, :C],
                    d["em"][:, :C],
                    mybir.ActivationFunctionType.Exp,
                )
                d["u"] = u_s

            def s_stt(i):
                bh, t = divmod(i, NT)
                C = P * (t + 1)
                d = st[i]
                rz2 = bh_state[bh][2]
                w_s = wpool.tile([P, S], BF16, tag="w", name=f"w_{i}")
                nc.vector.scalar_tensor_tensor(
                    out=w_s[:, :C],
                    in0=d["e"][:, :C],
                    scalar=d["top8"][:, KI - 1 : KI],
                    in1=d["u"][:, :C],
                    op0=mybir.AluOpType.is_ge,
                    op1=mybir.AluOpType.mult,
                    accum_out=rz2[:, t : t + 1],
                )
                if t == 0:
                    nc.vector.tensor_scalar_add(
                        rz2[0:KI, 0:1], rz2[0:KI, 0:1], float(S - P)
                    )
                d["w"] = w_s

            def s_tr_alloc(i):
                d = st[i]
                d["wtp"] = ps_wt.tile([P, NT, P], BF16, tag="wtp", name=f"wtp_{i}")

            def s_tr_chunk(i, c):
                d = st[i]
                nc.tensor.transpose(
                    d["wtp"][:, c, :], d["w"][:, bass.ts(c, P)], ident_b[:, :]
                )

            def s_ev(i):
                bh, t = divmod(i, NT)
                d = st[i]
                wt_s = wtpool.tile([P, NT, P], BF16, tag="wt", name=f"wt_{i}")
                nc.vector.tensor_copy(wt_s[:, 0 : t + 1, :], d["wtp"][:, 0 : t + 1, :])
                d["wt"] = wt_s

            def s_mm2_chunk(i, c):
                bh, t = divmod(i, NT)
                d = st[i]
                qk, _, _, o_ps_t, half = bh_state[bh]
                v0 = 3 * S  # fp16-col offset of the bf16 V block
                nc.tensor.matmul(
                    o_ps_t[:, t, :],
                    lhsT=d["wt"][:, c, :],
                    rhs=qk[:, v0 + c * VW : v0 + (c + 1) * VW].bitcast(BF16),
                    start=(c == 0),
                    stop=(c == t and t > 0),
                )
                if t == 0:
                    for cc in range(1, NT):
                        nc.tensor.matmul(
                            o_ps_t[0:KI, 0, :],
                            lhsT=ones_k[:, 0:KI],
                            rhs=qk[:, v0 + cc * VW : v0 + (cc + 1) * VW].bitcast(BF16),
                            start=False,
                            stop=(cc == NT - 1),
                        )

            def s_osc(i):
                bh, t = divmod(i, NT)
                st.pop(i)
                if t != NT - 1:
                    return
                qk, o_all, rz2, o_ps_t, half = bh_state[bh]
                nc.vector.reciprocal(rz2[:, :], rz2[:, :])
                nc.vector.memset(rz2[0:1, 0:1], 0.0)
                nc.vector.tensor_tensor(
                    out=o_all[:, :, :],
                    in0=o_ps_t[:, :, 0:D],
                    in1=rz2[:, :, None].broadcast_to([P, NT, D]),
                    op=mybir.AluOpType.mult,
                )
                nc.sync.dma_start(
                    out[bh].rearrange("(c p) d -> p c d", p=P), o_all[:, :, :]
                )
                del bh_state[bh]

            # G=2 head interleave, stage-major emission so paired heads'
            # PE ops alternate PSUM banks (non-conflicting writes stream
            # 2x) and the Tile scheduler can pipeline across engines.
            G = 4
            for g0 in range(0, bh_count, G):
                members = list(range(g0, min(g0 + G, bh_count)))
                for bh in members:
                    s_dma(bh * NT)
                for t in range(NT):
                    ii = [bh * NT + t for bh in members]
                    for fn in (s_mm1, s_exp1, s_top8, s_em, s_exp2, s_stt):
                        for i in ii:
                            fn(i)
                    # tr/ev/mm2 in sub-pairs: wtp pool has 2 bufs, so only
                    # two heads' transposes can alternate PSUM banks at once
                    for sub in (ii[:2], ii[2:]):
                        for i in sub:
                            s_tr_alloc(i)
                        for c in range(t + 1):
                            for i in sub:
                                s_tr_chunk(i, c)
                        for i in sub:
                            s_ev(i)
                        for c in range(t + 1):
                            for i in sub:
                                s_mm2_chunk(i, c)
                    for i in ii:
                        s_osc(i)
    nc.compile()
    return nc


def _get_nc(bh_count, S, D, d_k, k_index):
    key = (bh_count, S, D, d_k, k_index)
    if key not in _NC_CACHE:
        _NC_CACHE[key] = _build(bh_count, S, D, d_k, k_index)
    return _NC_CACHE[key]


def _numpy_fallback(q, k, v, mask, d_k, k_index):
    """Straight port of the reference for inputs the Bass kernel doesn't
    cover (non-causal mask / incompatible shapes). Slow but correct."""
    NEG = np.float32(-1e32)
    b, h, s, _ = q.shape
    scores = np.einsum("bhqd,bhkd->bhqk", q, k) / np.sqrt(np.float32(d_k))
    scores = np.where(mask == 0, NEG, scores)
    scores = scores - scores.max(axis=-1, keepdims=True)
    e = np.exp(scores)
    scores = e / e.sum(axis=-1, keepdims=True)
    sa = scores[:, :, :k_index, :]
    sb = scores[:, :, k_index:, :].reshape(b * h * (s - k_index), s)
    srt = -np.sort(-sb, axis=-1)
    thr = srt[:, k_index - 1 : k_index]
    sb = np.where(sb - thr >= 0, sb, NEG)
    sb = sb.reshape(b, h, s - k_index, s)
    scores = np.concatenate([sa, sb], axis=2)
    scores = scores - scores.max(axis=-1, keepdims=True)
    e = np.exp(scores)
    scores = e / e.sum(axis=-1, keepdims=True)
    scores[:, :, 0, :] = 0.0
    return np.einsum("bhqk,bhkd->bhqd", scores, v).astype(np.float32)


def _is_causal(mask, S):
    if mask is None:
        return True
    m = np.asarray(mask)
    if m.size != S * S:
        return False
    return bool(np.array_equal(m.reshape(S, S) != 0, np.tril(np.ones((S, S), bool))))


def kernel(q, k, v, mask=None, d_k=None, k_index=None, **_unused):
    global LAST_RESULT
    q = np.asarray(q, dtype=np.float32)
    k = np.asarray(k, dtype=np.float32)
    v = np.asarray(v, dtype=np.float32)
    B, H, S, D = q.shape
    d_k = int(d_k) if d_k is not None else D
    k_index = int(k_index) if k_index is not None else 5

    if (
        B % N_CORES != 0
        or S % 128 != 0
        or D != 64
        or not (1 <= k_index <= 8)
        or not _is_causal(mask, S)
    ):
        mask_np = (
            np.asarray(mask)
            if mask is not None
            else np.tril(np.ones((S, S), np.int32))[None, None]
        )
        return _numpy_fallback(q, k, v, mask_np, d_k, k_index)

    P = 128
    NT = S // P
    bpc = B // N_CORES
    bh_full = bpc * H
    bh_count = BH_OVERRIDE or bh_full

    qT = np.transpose(q, (0, 1, 3, 2))  # [B, H, D, S]
    kT = np.transpose(k, (0, 1, 3, 2))
    qh = qT.astype(np.float16)
    ql = (qT - qh.astype(np.float32)).astype(np.float16)
    kh = kT.astype(np.float16)
    kl = (kT - kh.astype(np.float32)).astype(np.float16)
    # fp16 rows packed [qh;ql | kh;kh | kl;kl]: [B,H,128,3S] viewed per-bh
    qk16 = np.concatenate(
        [
            np.concatenate([qh, ql], axis=2),
            np.concatenate([kh, kh], axis=2),
            np.concatenate([kl, kl], axis=2),
        ],
        axis=3,
    )  # [B, H, 2D=128, 3S]
    # bf16 [V|1|0] block: [B,H,128,NT*VW]
    vb = np.zeros((B, H, P, NT * VW), dtype=ml_dtypes.bfloat16)
    vr = v.reshape(B, H, NT, P, D).transpose(0, 1, 3, 2, 4)  # [B,H,P,NT,D]
    vbl = vb.reshape(B, H, P, NT, VW)
    vbl[..., 0:D] = vr.astype(ml_dtypes.bfloat16)
    qkv_h = np.concatenate(
        [qk16.view(np.uint16), vb.view(np.uint16).reshape(B, H, P, NT * VW)],
        axis=3,
    ).view(np.float16)  # [B, H, 128, 3S + NT*VW]

    nc = _get_nc(bh_count, S, D, d_k, k_index)

    in_maps = []
    for i in range(N_CORES):
        sl = slice(i * bpc, (i + 1) * bpc)
        in_maps.append(
            {
                "qkv": np.ascontiguousarray(
                    qkv_h[sl].reshape(bh_full, P, qkv_h.shape[3])[:bh_count]
                ),
            }
        )

    res = run_bass_kernel_spmd(
        nc, in_maps, core_ids=list(range(N_CORES)), trace=TRACE
    )
    LAST_RESULT = res

    outs = [
        np.asarray(res.results[i]["out"], dtype=np.float32) for i in range(N_CORES)
    ]
    if bh_count != bh_full:
        outs = [
            np.concatenate(
                [o, np.zeros((bh_full - bh_count, S, D), np.float32)], axis=0
            )
            for o in outs
        ]
    return np.concatenate([o.reshape(bpc, H, S, D) for o in outs], axis=0)


# revision 6
# speedup vs baseline: 1.1327x; 1.1085x over previous
"""Trainium2 Bass kernel v2: sparse (top-k) causal attention, data-parallel over batch.

Reference semantics (B=32, H=8, S=512, D=64, k_index=5):
  S_raw = (Q @ K^T) / sqrt(d_k), causal-masked
  P     = softmax(S_raw)
  rows >= k_index: keep only P >= (k_index-th largest of row)
  W     = softmax(P');  W[row 0] = 0;  out = W @ V

Key structure (per core: 32 heads x 4 causal q-tiles of 128 rows):
  - QK^T as two K=128 fp16 matmuls: [qh;ql]@[kh;kh] + [qh;ql]@[kl;kl]
    (q=qh+ql, k=kh+kl fp16 splits -> ~2^-22 score accuracy at 16-bit PE
    rate). First matmul start=True: non-accumulating PSUM writes stream
    2x faster than accumulating ones; mask matmul accumulates last.
  - e = exp(s/8) on ACT with row-sum z accumulated (no max-subtraction
    needed; scores ~ N(0,1)).
  - em = e/z via gpsimd normalize_recip (the only fast Pool op), so the
    second exp runs at full ACT rate with immediate scale (tensor-scale
    activations run at half rate).
  - u = exp(em); w = (e >= top8[k-1]) * u on DVE -> bf16.
  - W^T via PE transposes, stage-interleaved across the head pair so
    consecutive transposes alternate PSUM banks (56ns vs 107ns).
  - mm2 rhs = [V | 1]: the ones column makes each o_ps row also carry
    z2 = sum_kept(u), including the (S-128) tail mass for rows<k_index
    via the rank-k ones matmuls. One reciprocal + one broadcasted
    tensor_tensor per head scales all 4 tiles at once.
  - rows < k_index keep everything (thr=-1); row 0 zeroed via rz2=0.

Sharding: batch 32 -> 4 per core across 8 cores; each (b,h) independent.
Host packs one [bh, 128, 1792] uint16 tile per head: fp16 [qh;ql],
[kh;kh], [kl;kl] pre-transposed plus bf16 [V|1|0] -> single input DMA.
"""

import math

import numpy as np
import ml_dtypes

import concourse.bass as bass
import concourse.bacc as bacc
import concourse.mybir as mybir
import concourse.tile as tile
from concourse.bass_utils import run_bass_kernel_spmd
from concourse.masks import make_causal_mask, make_identity

N_CORES = 8
F32 = mybir.dt.float32
BF16 = mybir.dt.bfloat16
FP16 = mybir.dt.float16

# test.py hooks
TRACE = False
LAST_RESULT = None
BH_OVERRIDE = None  # dev only: limit (b,h) pairs per core

_NC_CACHE = {}

VW = 64  # V columns (z2 comes from the stt accumulator)


def _build(bh_count: int, S: int, D: int, d_k: int, k_index: int) -> bass.Bass:
    P = 128
    NT = S // P
    KI = k_index
    NEG = -1.0e5
    scale = 1.0 / math.sqrt(float(d_k))
    assert 1 <= KI <= 8 and S % P == 0 and D <= P

    QKV_W = 3 * S + NT * VW  # fp16 cols: qh|ql, kh|kh, kl|kl, then bf16 [V|1|0]

    nc = bacc.Bacc("TRN2", target_bir_lowering=False, debug=False)
    qkv = nc.declare_dram_parameter("qkv", [bh_count, P, QKV_W], FP16, isOutput=False)
    out = nc.declare_dram_parameter("out", [bh_count, S, D], F32, isOutput=True)

    with tile.TileContext(nc) as tc:
        with (
            tc.tile_pool(name="const", bufs=1) as cpool,
            tc.tile_pool(name="inp", bufs=8) as ipool,
            tc.tile_pool(name="big", bufs=8) as bpool,
            tc.tile_pool(name="wbuf", bufs=8) as wpool,
            tc.tile_pool(name="wt", bufs=8) as wtpool,
            tc.tile_pool(name="stat", bufs=12) as spool,
            tc.tile_pool(name="obuf", bufs=5) as opool,
            tc.tile_pool(name="ps_s", bufs=2, space="PSUM") as ps_s,
            tc.tile_pool(name="ps_o", bufs=4, space="PSUM") as ps_o,
            tc.tile_pool(name="ps_wt", bufs=2, space="PSUM") as ps_wt,
        ):
            # constants
            mask_f = cpool.tile([P, P], F32)
            make_causal_mask(nc, mask_f[:, :], mask_val=NEG)
            mask_b = cpool.tile([P, P], BF16)
            nc.vector.tensor_copy(mask_b[:, :], mask_f[:, :])
            ident_f = cpool.tile([P, P], F32)
            make_identity(nc, ident_f[:, :])
            ident_b = cpool.tile([P, P], BF16)
            nc.vector.tensor_copy(ident_b[:, :], ident_f[:, :])
            ones_k = cpool.tile([P, KI], BF16)
            nc.vector.memset(ones_k[:, :], 1.0)

            st = {}  # per-iteration tile state
            bh_state = {}  # per-head tiles

            def s_dma(i):
                bh, t = divmod(i, NT)
                if t:
                    return
                qk = ipool.tile([P, QKV_W], FP16, tag="qkv", name=f"qkv_{bh}")
                nc.sync.dma_start(qk[:, :], qkv[bh])
                o_all = opool.tile([P, NT, D], F32, tag="o_all", name=f"oall_{bh}")
                rz2 = spool.tile([P, NT], F32, tag="rz2", name=f"rz2_{bh}")
                o_ps_t = ps_o.tile([P, NT, VW], F32, tag="o", name=f"ops_{bh}")
                bh_state[bh] = (qk, o_all, rz2, o_ps_t, 0)

            def s_mm1(i):
                bh, t = divmod(i, NT)
                C = P * (t + 1)
                qk = bh_state[bh][0]
                s_ps = ps_s.tile([P, S], F32, tag="s", name=f"sps_{i}")
                # fresh (non-accumulating) write first: 2x PSUM rate
                nc.tensor.matmul(
                    s_ps[:, :C], lhsT=qk[:, bass.ts(t, P)],
                    rhs=qk[:, S : S + C], start=True, stop=False)
                nc.tensor.matmul(
                    s_ps[:, :C], lhsT=qk[:, bass.ts(t, P)],
                    rhs=qk[:, 2 * S : 2 * S + C], start=False, stop=False)
                nc.tensor.matmul(
                    s_ps[:, bass.ts(t, P)],
                    lhsT=ident_b[:, :], rhs=mask_b[:, :],
                    start=False, stop=True)
                st[i] = {"s_ps": s_ps}

            def s_exp1(i):
                bh, t = divmod(i, NT)
                C = P * (t + 1)
                d = st[i]
                e_s = bpool.tile([P, S], F32, tag="e", name=f"e_{i}")
                z = spool.tile([P, 1], F32, tag="z", name=f"z_{i}")
                nc.scalar.activation(
                    e_s[:, :C],
                    d["s_ps"][:, :C],
                    mybir.ActivationFunctionType.Exp,
                    scale=scale,
                    accum_out=z[:, :],
                )
                d["e"], d["z"] = e_s, z

            def s_top8(i):
                bh, t = divmod(i, NT)
                C = P * (t + 1)
                d = st[i]
                top8 = spool.tile([P, 8], F32, tag="top8", name=f"top8_{i}")
                nc.vector.max(out=top8[:, :], in_=d["e"][:, :C])
                if t == 0:
                    nc.vector.memset(top8[0:KI, KI - 1 : KI], -1.0)
                d["top8"] = top8

            def s_em(i):
                bh, t = divmod(i, NT)
                C = P * (t + 1)
                d = st[i]
                em = bpool.tile([P, S], F32, tag="em", name=f"em_{i}")
                nc.gpsimd.normalize_recip(em[:, :C], d["e"][:, :C], d["z"][:, :])
                d["em"] = em

            def s_exp2(i):
                bh, t = divmod(i, NT)
                C = P * (t + 1)
                d = st[i]
                u_s = bpool.tile([P, S], F32, tag="u", name=f"u_{i}")
                nc.scalar.activation(
                    u_s[:, :C],
                    d["em"][:, :C],
                    mybir.ActivationFunctionType.Exp,
                )
                d["u"] = u_s

            def s_stt(i):
                bh, t = divmod(i, NT)
                C = P * (t + 1)
                d = st[i]
                rz2 = bh_state[bh][2]
                w_s = wpool.tile([P, S], BF16, tag="w", name=f"w_{i}")
                nc.vector.scalar_tensor_tensor(
                    out=w_s[:, :C],
                    in0=d["e"][:, :C],
                    scalar=d["top8"][:, KI - 1 : KI],
                    in1=d["u"][:, :C],
                    op0=mybir.AluOpType.is_ge,
                    op1=mybir.AluOpType.mult,
                    accum_out=rz2[:, t : t + 1],
                )
                if t == 0:
                    nc.vector.tensor_scalar_add(
                        rz2[0:KI, 0:1], rz2[0:KI, 0:1], float(S - P)
                    )
                d["w"] = w_s

            def s_tr_alloc(i):
                d = st[i]
                d["wtp"] = ps_wt.tile([P, NT, P], BF16, tag="wtp", name=f"wtp_{i}")

            def s_tr_chunk(i, c):
                d = st[i]
                nc.tensor.transpose(
                    d["wtp"][:, c, :], d["w"][:, bass.ts(c, P)], ident_b[:, :]
                )

            def s_ev(i):
                bh, t = divmod(i, NT)
                d = st[i]
                wt_s = wtpool.tile([P, NT, P], BF16, tag="wt", name=f"wt_{i}")
                if t == 0:
                    # smallest evict off the bottleneck DVE engine
                    nc.scalar.copy(wt_s[:, 0:1, :], d["wtp"][:, 0:1, :])
                else:
                    nc.vector.tensor_copy(
                        wt_s[:, 0 : t + 1, :], d["wtp"][:, 0 : t + 1, :]
                    )
                d["wt"] = wt_s

            def s_mm2_chunk(i, c):
                bh, t = divmod(i, NT)
                d = st[i]
                qk, _, _, o_ps_t, half = bh_state[bh]
                v0 = 3 * S  # fp16-col offset of the bf16 V block
                nc.tensor.matmul(
                    o_ps_t[:, t, :],
                    lhsT=d["wt"][:, c, :],
                    rhs=qk[:, v0 + c * VW : v0 + (c + 1) * VW].bitcast(BF16),
                    start=(c == 0),
                    stop=(c == t and t > 0),
                )
                if t == 0:
                    for cc in range(1, NT):
                        nc.tensor.matmul(
                            o_ps_t[0:KI, 0, :],
                            lhsT=ones_k[:, 0:KI],
                            rhs=qk[:, v0 + cc * VW : v0 + (cc + 1) * VW].bitcast(BF16),
                            start=False,
                            stop=(cc == NT - 1),
                        )

            def s_osc(i):
                bh, t = divmod(i, NT)
                st.pop(i)
                if t != NT - 1:
                    return
                qk, o_all, rz2, o_ps_t, half = bh_state[bh]
                nc.vector.reciprocal(rz2[:, :], rz2[:, :])
                nc.vector.memset(rz2[0:1, 0:1], 0.0)
                nc.vector.tensor_tensor(
                    out=o_all[:, :, :],
                    in0=o_ps_t[:, :, 0:D],
                    in1=rz2[:, :, None].broadcast_to([P, NT, D]),
                    op=mybir.AluOpType.mult,
                )
                nc.sync.dma_start(
                    out[bh].rearrange("(c p) d -> p c d", p=P), o_all[:, :, :]
                )
                del bh_state[bh]

            # G=2 head interleave, stage-major emission so paired heads'
            # PE ops alternate PSUM banks (non-conflicting writes stream
            # 2x) and the Tile scheduler can pipeline across engines.
            G = 4
            for g0 in range(0, bh_count, G):
                members = list(range(g0, min(g0 + G, bh_count)))
                for bh in members:
                    s_dma(bh * NT)
                for t in range(NT):
                    ii = [bh * NT + t for bh in members]
                    for fn in (s_mm1, s_exp1, s_top8, s_em, s_exp2, s_stt):
                        for i in ii:
                            fn(i)
                    # tr/ev/mm2 in sub-pairs: wtp pool has 2 bufs, so only
                    # two heads' transposes can alternate PSUM banks at once
                    for sub in (ii[:2], ii[2:]):
                        for i in sub:
                            s_tr_alloc(i)
                        for c in range(t + 1):
                            for i in sub:
                                s_tr_chunk(i, c)
                        for i in sub:
                            s_ev(i)
                        for c in range(t + 1):
                            for i in sub:
                                s_mm2_chunk(i, c)
                    for i in ii:
                        s_osc(i)
    nc.compile()
    return nc


def _get_nc(bh_count, S, D, d_k, k_index):
    key = (bh_count, S, D, d_k, k_index)
    if key not in _NC_CACHE:
        _NC_CACHE[key] = _build(bh_count, S, D, d_k, k_index)
    return _NC_CACHE[key]


def _numpy_fallback(q, k, v, mask, d_k, k_index):
    """Straight port of the reference for inputs the Bass kernel doesn't
    cover (non-causal mask / incompatible shapes). Slow but correct."""
    NEG = np.float32(-1e32)
    b, h, s, _ = q.shape
    scores = np.einsum("bhqd,bhkd->bhqk", q, k) / np.sqrt(np.float32(d_k))
    scores = np.where(mask == 0, NEG, scores)
    scores = scores - scores.max(axis=-1, keepdims=True)
    e = np.exp(scores)
    scores = e / e.sum(axis=-1, keepdims=True)
    sa = scores[:, :, :k_index, :]
    sb = scores[:, :, k_index:, :].reshape(b * h * (s - k_index), s)
    srt = -np.sort(-sb, axis=-1)
    thr = srt[:, k_index - 1 : k_index]
    sb = np.where(sb - thr >= 0, sb, NEG)
    sb = sb.reshape(b, h, s - k_index, s)
    scores = np.concatenate([sa, sb], axis=2)
    scores = scores - scores.max(axis=-1, keepdims=True)
    e = np.exp(scores)
    scores = e / e.sum(axis=-1, keepdims=True)
    scores[:, :, 0, :] = 0.0
    return np.einsum("bhqk,bhkd->bhqd", scores, v).astype(np.float32)


def _is_causal(mask, S):
    if mask is None:
        return True
    m = np.asarray(mask)
    if m.size != S * S:
        return False
    return bool(np.array_equal(m.reshape(S, S) != 0, np.tril(np.ones((S, S), bool))))


def kernel(q, k, v, mask=None, d_k=None, k_index=None, **_unused):
    global LAST_RESULT
    q = np.asarray(q, dtype=np.float32)
    k = np.asarray(k, dtype=np.float32)
    v = np.asarray(v, dtype=np.float32)
    B, H, S, D = q.shape
    d_k = int(d_k) if d_k is not None else D
    k_index = int(k_index) if k_index is not None else 5

    if (
        B % N_CORES != 0
        or S % 128 != 0
        or D != 64
        or not (1 <= k_index <= 8)
        or not _is_causal(mask, S)
    ):
        mask_np = (
            np.asarray(mask)
            if mask is not None
            else np.tril(np.ones((S, S), np.int32))[None, None]
        )
        return _numpy_fallback(q, k, v, mask_np, d_k, k_index)

    P = 128
    NT = S // P
    bpc = B // N_CORES
    bh_full = bpc * H
    bh_count = BH_OVERRIDE or bh_full

    qT = np.transpose(q, (0, 1, 3, 2))  # [B, H, D, S]
    kT = np.transpose(k, (0, 1, 3, 2))
    qh = qT.astype(np.float16)
    ql = (qT - qh.astype(np.float32)).astype(np.float16)
    kh = kT.astype(np.float16)
    kl = (kT - kh.astype(np.float32)).astype(np.float16)
    # fp16 rows packed [qh;ql | kh;kh | kl;kl]: [B,H,128,3S] viewed per-bh
    qk16 = np.concatenate(
        [
            np.concatenate([qh, ql], axis=2),
            np.concatenate([kh, kh], axis=2),
            np.concatenate([kl, kl], axis=2),
        ],
        axis=3,
    )  # [B, H, 2D=128, 3S]
    # bf16 [V|1|0] block: [B,H,128,NT*VW]
    vb = np.zeros((B, H, P, NT * VW), dtype=ml_dtypes.bfloat16)
    vr = v.reshape(B, H, NT, P, D).transpose(0, 1, 3, 2, 4)  # [B,H,P,NT,D]
    vbl = vb.reshape(B, H, P, NT, VW)
    vbl[..., 0:D] = vr.astype(ml_dtypes.bfloat16)
    qkv_h = np.concatenate(
        [qk16.view(np.uint16), vb.view(np.uint16).reshape(B, H, P, NT * VW)],
        axis=3,
    ).view(np.float16)  # [B, H, 128, 3S + NT*VW]

    nc = _get_nc(bh_count, S, D, d_k, k_index)

    in_maps = []
    for i in range(N_CORES):
        sl = slice(i * bpc, (i + 1) * bpc)
        in_maps.append(
            {
                "qkv": np.ascontiguousarray(
                    qkv_h[sl].reshape(bh_full, P, qkv_h.shape[3])[:bh_count]
                ),
            }
        )

    res = run_bass_kernel_spmd(
        nc, in_maps, core_ids=list(range(N_CORES)), trace=TRACE
    )
    LAST_RESULT = res

    outs = [
        np.asarray(res.results[i]["out"], dtype=np.float32) for i in range(N_CORES)
    ]
    if bh_count != bh_full:
        outs = [
            np.concatenate(
                [o, np.zeros((bh_full - bh_count, S, D), np.float32)], axis=0
            )
            for o in outs
        ]
    return np.concatenate([o.reshape(bpc, H, S, D) for o in outs], axis=0)
